# revision 9
# baseline (speedup 1.0000x reference)
"""GQA kernel for Trainium2: 8-core tensor-parallel (heads sharded).

Problem (hardcoded): B=1, S=2048, D=2048, H=32 q-heads, KVH=8 kv-heads, HD=64.
Core c owns q-heads 4c..4c+3 and kv-head c.

Per-core dataflow:
  1. PE-transpose x -> xT [D(part), S] in SBUF.
  2. Projections (contract D): QT per head-pair [128, S], KT [64, S] (duplicated
     to partitions 64-127 so odd heads can run base-64 matmuls), V natural
     [S(part), 64] with an appended ones column (folds the softmax denominator
     into the P.V matmul).
  3. Per (q-block, head): scoresT [k(part), q] = KT.T @ QT on PE; exp on ACT
     (scale=1/8 folded in); P.V matmul outT[65, q] (row 64 = sum of exps);
     recip via ACT Ln/Exp; normalize exp tiles on DVE (bf16 2x); DMA attn out
     in [k, q] layout (host transposes the view).
  4. outT (normalized) -> Wo matmul -> per-core partial output; host sums.
"""

import numpy as np

B, S, D = 1, 2048, 2048
H, KVH, HD = 32, 8, 64
G = H // KVH            # 4 q-heads per kv head = per core
NC_CORES = 8
P = 128
DC = D // P             # 16 contraction chunks
ST = S // P             # 16 sequence tiles
QW = 512                # q block width
QB = S // QW            # 4 q blocks
KT = S // P             # 16 k tiles

CFG = {
    "mm_dt": "float32r",     # matmul input mode for fp32-stored tiles
    "attn_dt": "bfloat16",   # exp/attn storage + output dtype
}

_CACHE = {}
TRACE = False       # test-only: capture NTFF profile via run_bass_kernel_spmd
_last_perf = None


def _build_nc():
    import concourse.bacc as bacc
    import concourse.mybir as mybir
    import concourse.tile as tile
    from concourse.masks import make_identity

    f32 = mybir.dt.float32
    mm_dt = getattr(mybir.dt, CFG["mm_dt"])
    adt = getattr(mybir.dt, CFG["attn_dt"])
    AF = mybir.ActivationFunctionType
    Alu = mybir.AluOpType

    def mc(ap):
        return ap.bitcast(mm_dt) if mm_dt != f32 else ap

    def dview(ap):
        # DRAM-side reinterpret so fp32 weights load into fp32r tiles
        return ap.bitcast(mm_dt) if mm_dt != f32 else ap

    nc = bacc.Bacc("TRN2", target_bir_lowering=False)

    x_d = nc.dram_tensor("x", [S, D], f32, kind="ExternalInput")
    wq_d = nc.dram_tensor("wq", [D, G * HD], f32, kind="ExternalInput")
    wk_d = nc.dram_tensor("wk", [D, HD], f32, kind="ExternalInput")
    wv_d = nc.dram_tensor("wv", [D, HD], f32, kind="ExternalInput")
    wo_d = nc.dram_tensor("wo", [G * HD, D], f32, kind="ExternalInput")
    attn_d = nc.dram_tensor("attn_t", [G, S, S], adt, kind="ExternalOutput")
    pout_d = nc.dram_tensor("pout", [S, D], f32, kind="ExternalOutput")

    with tile.TileContext(nc) as tc:
        with (
            tc.tile_pool(name="const", bufs=1) as const,
            tc.tile_pool(name="persist", bufs=1) as persist,
        ):
            ident = const.tile([P, P], f32)
            make_identity(nc, ident)

            # Persistent SBUF state
            qt_pairs = persist.tile([P, 2, S], mm_dt)     # 16KB/part
            kt_dual = persist.tile([P, S], mm_dt)         # 8KB/part (K dup'd hi/lo)
            vone = persist.tile([P, KT, HD + 1], adt)   # ones col + V natural
            nc.vector.memset(vone[:, :, HD : HD + 1], 1.0)
            ones_row = const.tile([1, P], f32)
            nc.vector.memset(ones_row, 1.0)

            # ---------------- Phase A: transpose x ----------------
            with (
                tc.tile_pool(name="xtp", bufs=1) as xtp,
                tc.tile_pool(name="xload", bufs=6) as xload,
                tc.tile_pool(name="wpool", bufs=1) as wpool,
                tc.tile_pool(name="tpsum", bufs=2, space="PSUM") as tpsum,
                tc.tile_pool(name="ppsum", bufs=2, space="PSUM") as ppsum,
                tc.tile_pool(name="vpsum", bufs=2, space="PSUM") as vpsum,
            ):
                xt = xtp.tile([P, DC, S], mm_dt)          # 128KB/part
                for so in range(ST):
                    for dg in range(4):                 # groups of 4 d-chunks
                        xs = xload.tile([P, 4 * P], f32, tag="xs")
                        nc.sync.dma_start(
                            xs,
                            x_d[so * P : (so + 1) * P, dg * 512 : (dg + 1) * 512],
                        )
                        ps = tpsum.tile([P, 4 * P], f32, tag="tp")
                        for j in range(4):
                            nc.tensor.transpose(
                                ps[:, j * P : (j + 1) * P],
                                xs[:, j * P : (j + 1) * P],
                                ident,
                            )
                        nc.any.tensor_copy(
                            out=xt[:, dg * 4 : (dg + 1) * 4, so * P : (so + 1) * P],
                            in_=ps.rearrange("p (j s) -> p j s", j=4),
                        )

                # ---------------- Phase B: projections ----------------
                # Q: per pair of heads, M=128
                for pr in range(2):
                    wq_t = wpool.tile([P, DC, P], mm_dt, tag="wq")
                    nc.sync.dma_start(
                        wq_t,
                        dview(wq_d.rearrange("(dc p) m -> p dc m", p=P)[
                            :, :, pr * P : (pr + 1) * P
                        ]),
                    )
                    for qb in range(QB):
                        pq = ppsum.tile([P, QW], f32, tag="proj")
                        for dc in range(DC):
                            nc.tensor.matmul(
                                pq,
                                wq_t[:, dc, :],
                                xt[:, dc, qb * QW : (qb + 1) * QW],
                                start=(dc == 0),
                                stop=(dc == DC - 1),
                            )
                        nc.any.tensor_copy(
                            out=qt_pairs[:, pr, qb * QW : (qb + 1) * QW], in_=pq
                        )

                # K: M=64
                wk_t = wpool.tile([P, DC, HD], mm_dt, tag="wk")
                nc.sync.dma_start(wk_t, dview(wk_d.rearrange("(dc p) m -> p dc m", p=P)))
                for qb in range(QB):
                    pk = ppsum.tile([P, QW], f32, tag="proj")
                    for dc in range(DC):
                        nc.tensor.matmul(
                            pk[0:HD, :],
                            wk_t[:, dc, :],
                            xt[:, dc, qb * QW : (qb + 1) * QW],
                            start=(dc == 0),
                            stop=(dc == DC - 1),
                        )
                    nc.any.tensor_copy(
                        out=kt_dual[0:HD, qb * QW : (qb + 1) * QW], in_=pk[0:HD, :]
                    )
                # duplicate K rows to partitions 64-127
                nc.sync.dma_start(kt_dual[HD : 2 * HD, :], kt_dual[0:HD, :])

                # V natural: out[s, hd], lhsT = xT chunk, rhs = Wv chunk
                wv_t = wpool.tile([P, DC, HD], mm_dt, tag="wv")
                nc.sync.dma_start(wv_t, dview(wv_d.rearrange("(dc p) m -> p dc m", p=P)))
                for st in range(ST):
                    pv = vpsum.tile([P, HD], f32, tag="vnat")
                    for dc in range(DC):
                        nc.tensor.matmul(
                            pv,
                            xt[:, dc, st * P : (st + 1) * P],
                            wv_t[:, dc, :],
                            start=(dc == 0),
                            stop=(dc == DC - 1),
                        )
                    nc.any.tensor_copy(out=vone[:, st, 0:HD], in_=pv)

            # ---------------- Phase C: attention + Wo ----------------
            with (
                tc.tile_pool(name="wop", bufs=1) as wop,
                tc.tile_pool(name="outp", bufs=1) as outp,
                tc.tile_pool(name="expp", bufs=2) as expp,
                tc.tile_pool(name="smallp", bufs=2) as smallp,
                tc.tile_pool(name="stagep", bufs=3) as stagep,
                tc.tile_pool(name="spsum", bufs=2, space="PSUM") as spsum,
                tc.tile_pool(name="pvpsum", bufs=2, space="PSUM") as pvpsum,
                tc.tile_pool(name="wpsum", bufs=2, space="PSUM") as wpsum,
            ):
                wo_t = wop.tile([P, 2, D], mm_dt)
                nc.sync.dma_start(wo_t, dview(wo_d.rearrange("(c p) n -> p c n", p=P)))
                out_t = outp.tile([P, 2, S], mm_dt)   # normalized attn-out^T

                for qb in range(QB):
                    qsl = slice(qb * QW, (qb + 1) * QW)
                    for h in range(G):
                        pr, par = h // 2, h % 2
                        base = HD * par
                        ex = expp.tile([P, KT, QW], adt, tag="ex")
                        for kg in range(KT // 2):
                            sp = spsum.tile([P, 2 * QW], f32, tag="sc")
                            for j in range(2):
                                kt = kg * 2 + j
                                nc.tensor.matmul(
                                    sp[:, j * QW : (j + 1) * QW],
                                    kt_dual[base : base + HD, kt * P : (kt + 1) * P],
                                    qt_pairs[base : base + HD, pr, qsl],
                                    start=True,
                                    stop=True,
                                )
                            nc.scalar.activation(
                                ex[:, kg * 2 : kg * 2 + 2, :],
                                sp.rearrange("p (j q) -> p j q", j=2),
                                AF.Exp,
                                scale=1.0 / 8.0,
                            )
                        # P.V, ones column last: rows 0..63 = unnormalized
                        # outT, row 64 = sum of exps (quadrant-aligned reads)
                        po = pvpsum.tile([HD + 1, QW], f32, tag="pv")
                        for kt in range(KT):
                            nc.tensor.matmul(
                                po,
                                vone[:, kt, :],
                                ex[:, kt, :],
                                start=(kt == 0),
                                stop=(kt == KT - 1),
                            )
                        # reciprocal of sums (row 64): r = exp(-ln(z))
                        lnz = smallp.tile([1, QW], f32, tag="lnz")
                        nc.scalar.activation(lnz, po[HD : HD + 1, :], AF.Ln)
                        r_f32 = smallp.tile([1, QW], f32, tag="rf32")
                        nc.scalar.activation(r_f32, lnz, AF.Exp, scale=-1.0)
                        # broadcast recip across partitions via K=1 matmul
                        bps = wpsum.tile([P, QW], f32, tag="wo")
                        nc.tensor.matmul(
                            bps, ones_row, r_f32, start=True, stop=True
                        )
                        rb = stagep.tile([P, QW], adt, tag="rb")
                        nc.scalar.copy(out=rb, in_=bps)
                        rf = stagep.tile([P, QW], f32, tag="rf")
                        nc.scalar.copy(out=rf, in_=bps)
                        # normalize exp tiles (bf16 2x) and write attn out
                        for kt in range(KT):
                            nc.vector.tensor_tensor(
                                ex[:, kt, :], ex[:, kt, :], rb, Alu.mult
                            )
                        nc.sync.dma_start(
                            attn_d[h].rearrange("(kt p) q -> p kt q", p=P)[:, :, qsl],
                            ex,
                        )
                        # normalized outT -> out_t (partition-realign via DMA)
                        st_o = stagep.tile([P, QW], mm_dt, tag="sto")
                        nc.vector.tensor_tensor(
                            st_o[0:HD, :],
                            po[0:HD, :],
                            rf[0:HD, :],
                            Alu.mult,
                        )
                        nc.sync.dma_start(
                            out_t[base : base + HD, pr, qsl], st_o[0:HD, :]
                        )

                    # Wo for this q-block's sequence rows
                    for j in range(QW // P):
                        stg = qb * (QW // P) + j
                        for nb in range(QB):
                            wps = wpsum.tile([P, QW], f32, tag="wo")
                            for c in range(2):
                                nc.tensor.matmul(
                                    wps,
                                    out_t[:, c, stg * P : (stg + 1) * P],
                                    wo_t[:, c, nb * QW : (nb + 1) * QW],
                                    start=(c == 0),
                                    stop=(c == 1),
                                )
                            pst = stagep.tile([P, QW], f32, tag="pst")
                            nc.any.tensor_copy(out=pst, in_=wps)
                            nc.sync.dma_start(
                                pout_d[stg * P : (stg + 1) * P, nb * QW : (nb + 1) * QW],
                                pst,
                            )

    nc.compile()
    return nc


def _get_nc():
    key = (CFG["mm_dt"], CFG["attn_dt"])
    if key not in _CACHE:
        _CACHE[key] = _build_nc()
    return _CACHE[key]


def kernel(x, Wq, Wk, Wv, Wo):
    from concourse.bass_utils import run_bass_kernel_spmd

    x = np.asarray(x, dtype=np.float32).reshape(S, D)
    Wq = np.asarray(Wq, dtype=np.float32)
    Wk = np.asarray(Wk, dtype=np.float32)
    Wv = np.asarray(Wv, dtype=np.float32)
    Wo = np.asarray(Wo, dtype=np.float32)

    in_maps = []
    for c in range(NC_CORES):
        qs = slice(c * G * HD, (c + 1) * G * HD)
        ks = slice(c * HD, (c + 1) * HD)
        in_maps.append(
            {
                "x": x,
                "wq": np.ascontiguousarray(Wq[:, qs]),
                "wk": np.ascontiguousarray(Wk[:, ks]),
                "wv": np.ascontiguousarray(Wv[:, ks]),
                "wo": np.ascontiguousarray(Wo[qs, :]),
            }
        )

    nc = _get_nc()
    res = run_bass_kernel_spmd(
        nc, in_maps, core_ids=list(range(NC_CORES)), trace=TRACE
    )
    global _last_perf
    _last_perf = res
    results = res.results

    output = np.zeros((S, D), dtype=np.float32)
    for c in range(NC_CORES):
        output += results[c]["pout"]

    attn = np.empty((B, H, S, S), dtype=np.float32)
    for hh in range(H):
        c, g = hh // G, hh % G
        attn[0, hh] = results[c]["attn_t"][g].T.astype(np.float32)

    return output.reshape(B, S, D), attn


if __name__ == "__main__":
    nc = _build_nc()
    print("build OK; instructions:",
          sum(len(b.instructions) for f in nc.m.functions for b in f.blocks)
          if hasattr(nc.m.functions[0], "blocks") else "n/a")
